# revision 10
# baseline (speedup 1.0000x reference)
"""Trainium2 Bass kernel for nn_ChannelCompressAttention.

Shapes: x (8, 4096, 1024) f32, w_qkv (3072, 1024) f32, w_conv1 (1024,) f32.
Output: (8, 4096, 1024) f32.

Math: with q,k,v = split(x @ w_qkv^T), agent = q @ w_conv1,
  aa   = softmax_c(scale * agent @ k)          # (c,)
  p    = softmax_n(aa @ v^T)                   # (n,)
  out  = softmax(agent[:,:,None], -1) * (p @ v)[None]   # softmax over a
                                                        # singleton axis == 1
so every row of the output equals agent_v = p @ v.  All uses of q/k/v are
rank-1 contractions, so the 3c x c projection never needs to be
materialized:
  u  = scale * Wq^T w_conv1      agent = x u           (per batch)
  s  = x^T agent                 z     = Wk s
  aa = softmax(z)                t     = Wv^T aa
  sc = x t                       p     = softmax(sc)
  r  = x^T p                     out_row = Wv r
This drops the arithmetic from ~206 GFLOP to ~0.5 GFLOP; the kernel is
then HBM-bound (x in, 12 MB weights in, 16 MB out per core).

Sharding: data-parallel over batch, one batch per NeuronCore (8 cores).

On-core mapping (x kept resident in SBUF in natural (n-part, c-free)
layout):
  - contractions over n (s, r)  -> TensorE, lhsT = per-tile n-vector
  - contractions over c (agent, sc, z, out_row) -> VectorE fused
    tensor_tensor_reduce (multiply + free-dim reduce in one pass)
  - softmax partition sums      -> TensorE matmul with a ones vector
  - exp                         -> ScalarE (no max subtraction needed:
    logits are O(10) for this problem family, fp32 exp is safe)
Wk/Wv rows are loaded with an interleaved AP (row p*8+j on partition p of
tile j) so the final (128,8) -> (1,1024) reorder is a plain flat-order
SBUF->SBUF DMA.
"""

import sys

for _p in ("/opt/trn_rl_repo", "/opt/pypackages"):
    if _p not in sys.path:
        sys.path.insert(0, _p)

import numpy as np

import concourse.bass as bass
import concourse.bacc as bacc
import concourse.mybir as mybir
import concourse.tile as tile
from concourse.bass_utils import run_bass_kernel_spmd

B, N, C = 8, 4096, 1024
P = 128
NT = N // P          # 32 x-tiles per batch
J = C // P           # 8 weight tiles per matrix
F32 = mybir.dt.float32
SCALE = float(C) ** -0.5
H = 512              # max fp32 moving free dim per matmul


def _build():
    # Bacc (not raw Bass): its compile() pass splits multi-wait instructions
    # into event semaphores, which TRN2 codegen requires (1 wait/inst).
    nc = bacc.Bacc(None)
    xb = nc.declare_dram_parameter("xb", [N, C], F32, isOutput=False)
    wqkv = nc.declare_dram_parameter("w_qkv", [3 * C, C], F32, isOutput=False)
    wc = nc.declare_dram_parameter("w_conv1", [C], F32, isOutput=False)
    out = nc.declare_dram_parameter("out", [N, C], F32, isOutput=True)

    mult = mybir.AluOpType.mult
    add = mybir.AluOpType.add
    AF = mybir.ActivationFunctionType

    with tile.TileContext(nc) as tc:
        with (
            tc.tile_pool(name="xres", bufs=NT) as xpool,
            tc.tile_pool(name="wst", bufs=3) as wst,
            tc.tile_pool(name="wv", bufs=J) as wvpool,
            tc.tile_pool(name="bc", bufs=2) as bcpool,
            tc.tile_pool(name="scr", bufs=2) as scrpool,
            tc.tile_pool(name="vec", bufs=4) as vecpool,
            tc.tile_pool(name="rows", bufs=2) as rows,
            tc.tile_pool(name="small", bufs=1) as small,
            tc.tile_pool(name="pacc", bufs=4, space="PSUM") as pacc,
            tc.tile_pool(name="pbc", bufs=2, space="PSUM") as pbc,
            tc.tile_pool(name="pscal", bufs=2, space="PSUM") as pscal,
        ):
            ones_m = small.tile([1, P], F32, tag="ones_m")   # lhsT: bcast row->128p
            nc.vector.memset(ones_m, 1.0)
            ones_k = small.tile([P, 1], F32, tag="ones_k")   # rhs: partition sum
            nc.vector.memset(ones_k, 1.0)

            def psum_to_row(ps_lo, ps_hi, tag, scale=1.0):
                row = rows.tile([1, C], F32, tag="row")
                nc.scalar.activation(out=row[:, 0:H], in_=ps_lo, func=AF.Copy,
                                     scale=scale)
                nc.scalar.activation(out=row[:, H:C], in_=ps_hi, func=AF.Copy,
                                     scale=scale)
                return row

            def bcast_row(row):
                dest = bcpool.tile([P, C], F32, tag="bc")
                for h in range(2):
                    ps = pbc.tile([P, H], F32, tag="pbc")
                    nc.tensor.matmul(ps, lhsT=ones_m, rhs=row[:, h * H:(h + 1) * H],
                                     start=True, stop=True)
                    nc.scalar.activation(out=dest[:, h * H:(h + 1) * H], in_=ps,
                                         func=AF.Copy)
                return dest

            # w_conv1 as (128, 8): column j = contiguous d-chunk j
            wc_sb = small.tile([P, J], F32, tag="wc")
            nc.gpsimd.dma_start(out=wc_sb, in_=wc.rearrange("(j p) -> p j", p=P))

            # ---- u = scale * Wq^T w_conv1 ----
            u_lo = pacc.tile([1, H], F32, tag="acc")
            u_hi = pacc.tile([1, H], F32, tag="acc")
            for j in range(J):
                wq_j = wst.tile([P, C], F32, tag="w")
                nc.sync.dma_start(out=wq_j, in_=wqkv[j * P:(j + 1) * P, :])
                nc.tensor.matmul(u_lo, lhsT=wc_sb[:, j:j + 1], rhs=wq_j[:, 0:H],
                                 start=(j == 0), stop=(j == J - 1))
                nc.tensor.matmul(u_hi, lhsT=wc_sb[:, j:j + 1], rhs=wq_j[:, H:C],
                                 start=(j == 0), stop=(j == J - 1))
            u_bc = bcast_row(psum_to_row(u_lo, u_hi, "u_row", scale=SCALE))

            # ---- stream x; agent_i = x_i u (DVE), s += x_i^T agent_i (PE) ----
            x_tiles = []
            s_lo = pacc.tile([1, H], F32, tag="acc")
            s_hi = pacc.tile([1, H], F32, tag="acc")
            for i in range(NT):
                xt = xpool.tile([P, C], F32, tag="x")
                nc.sync.dma_start(out=xt, in_=xb[i * P:(i + 1) * P, :])
                x_tiles.append(xt)
                agent_i = vecpool.tile([P, 1], F32, tag="agent")
                scr = scrpool.tile([P, C], F32, tag="scr")
                nc.vector.scalar_tensor_tensor(
                    out=scr, in0=xt, scalar=1.0, in1=u_bc,
                    op0=mult, op1=mult, accum_out=agent_i)
                nc.tensor.matmul(s_lo, lhsT=agent_i, rhs=xt[:, 0:H],
                                 start=(i == 0), stop=(i == NT - 1))
                nc.tensor.matmul(s_hi, lhsT=agent_i, rhs=xt[:, H:C],
                                 start=(i == 0), stop=(i == NT - 1))
            s_bc = bcast_row(psum_to_row(s_lo, s_hi, "s_row"))

            # ---- z = Wk s (interleaved rows: tile j partition p = row p*8+j) ----
            wkb = wqkv[C:2 * C, :].rearrange("(p j) c -> j p c", j=J)
            z_col = small.tile([P, J], F32, tag="z")
            for j in range(J):
                wk_j = wst.tile([P, C], F32, tag="w")
                nc.sync.dma_start(out=wk_j, in_=wkb[j])
                scr = scrpool.tile([P, C], F32, tag="scr")
                nc.vector.scalar_tensor_tensor(
                    out=scr, in0=wk_j, scalar=1.0, in1=s_bc,
                    op0=mult, op1=mult, accum_out=z_col[:, j:j + 1])

            # ---- softmax over c (no max-sub; logits are O(10)) ----
            ez = small.tile([P, J], F32, tag="ez")
            ez_sum = small.tile([P, 1], F32, tag="ezs")
            nc.scalar.activation(out=ez, in_=z_col, func=AF.Exp, accum_out=ez_sum)
            z1 = pscal.tile([1, 1], F32, tag="pscal")
            nc.tensor.matmul(z1, lhsT=ez_sum, rhs=ones_k, start=True, stop=True)
            rz1 = small.tile([1, 1], F32, tag="rz1")
            nc.vector.reciprocal(out=rz1, in_=z1)

            # ---- t = Wv^T ez / Z1 (Wv resident, interleaved rows) ----
            wvb = wqkv[2 * C:3 * C, :].rearrange("(p j) c -> j p c", j=J)
            wv_tiles = []
            t_lo = pacc.tile([1, H], F32, tag="acc")
            t_hi = pacc.tile([1, H], F32, tag="acc")
            for j in range(J):
                wv_j = wvpool.tile([P, C], F32, tag="wv")
                nc.sync.dma_start(out=wv_j, in_=wvb[j])
                wv_tiles.append(wv_j)
                nc.tensor.matmul(t_lo, lhsT=ez[:, j:j + 1], rhs=wv_j[:, 0:H],
                                 start=(j == 0), stop=(j == J - 1))
                nc.tensor.matmul(t_hi, lhsT=ez[:, j:j + 1], rhs=wv_j[:, H:C],
                                 start=(j == 0), stop=(j == J - 1))
            t_bc = bcast_row(psum_to_row(t_lo, t_hi, "t_row", scale=rz1))

            # ---- sc_i = x_i t (DVE); ep_i = exp(sc_i) (ACT);
            #      r += x_i^T ep_i and Z2 += sum(ep_i) (PE) ----
            r_lo = pacc.tile([1, H], F32, tag="acc")
            r_hi = pacc.tile([1, H], F32, tag="acc")
            z2 = pscal.tile([1, 1], F32, tag="pscal")
            for i in range(NT):
                xt = x_tiles[i]
                sc_i = vecpool.tile([P, 1], F32, tag="sc")
                scr = scrpool.tile([P, C], F32, tag="scr")
                nc.vector.scalar_tensor_tensor(
                    out=scr, in0=xt, scalar=1.0, in1=t_bc,
                    op0=mult, op1=mult, accum_out=sc_i)
                ep_i = vecpool.tile([P, 1], F32, tag="ep")
                nc.scalar.activation(out=ep_i, in_=sc_i, func=AF.Exp)
                nc.tensor.matmul(r_lo, lhsT=ep_i, rhs=xt[:, 0:H],
                                 start=(i == 0), stop=(i == NT - 1))
                nc.tensor.matmul(r_hi, lhsT=ep_i, rhs=xt[:, H:C],
                                 start=(i == 0), stop=(i == NT - 1))
                nc.tensor.matmul(z2, lhsT=ep_i, rhs=ones_k,
                                 start=(i == 0), stop=(i == NT - 1))
            rz2 = small.tile([1, 1], F32, tag="rz2")
            nc.vector.reciprocal(out=rz2, in_=z2)
            r_bc = bcast_row(psum_to_row(r_lo, r_hi, "r_row", scale=rz2))

            # ---- out_row[p*8+j] = (Wv r)[p*8+j] ----
            vo_col = small.tile([P, J], F32, tag="vo")
            for j in range(J):
                scr = scrpool.tile([P, C], F32, tag="scr")
                nc.vector.scalar_tensor_tensor(
                    out=scr, in0=wv_tiles[j], scalar=1.0, in1=r_bc,
                    op0=mult, op1=mult, accum_out=vo_col[:, j:j + 1])
            # flat reorder (p,j) -> 1024-row; both APs iterate k = p*8+j
            vo_row = rows.tile([1, C], F32, tag="row")
            nc.gpsimd.dma_start(out=vo_row, in_=vo_col)
            ob = bcast_row(vo_row)
            for o in range(NT):
                nc.sync.dma_start(out=out[o * P:(o + 1) * P, :], in_=ob)

    return nc


_CACHE = {}


def _get_nc():
    if "nc" not in _CACHE:
        nc = _build()
        # Bacc defers register allocation etc. to finalize-time compile();
        # run_bass_via_pjrt ships the BIR as-is, so finalize here.
        nc.finalize()
        _CACHE["nc"] = nc
    return _CACHE["nc"]


def _in_maps(x, w_qkv, w_conv1):
    return [{"xb": x[b], "w_qkv": w_qkv, "w_conv1": w_conv1} for b in range(B)]


def run(x, w_qkv, w_conv1, **spmd_kwargs):
    x = np.ascontiguousarray(np.asarray(x, dtype=np.float32))
    w_qkv = np.ascontiguousarray(np.asarray(w_qkv, dtype=np.float32))
    w_conv1 = np.ascontiguousarray(np.asarray(w_conv1, dtype=np.float32))
    res = run_bass_kernel_spmd(_get_nc(), _in_maps(x, w_qkv, w_conv1),
                               list(range(B)), **spmd_kwargs)
    out = np.stack([res.results[b]["out"] for b in range(B)], axis=0)
    return out, res


def kernel(x, w_qkv, w_conv1):
    out, _ = run(x, w_qkv, w_conv1)
    return out


if __name__ == "__main__":
    rng = np.random.default_rng(0)
    x = rng.standard_normal((B, N, C), dtype=np.float32)
    w_qkv = rng.standard_normal((3 * C, C), dtype=np.float32) * SCALE
    w_conv1 = rng.standard_normal(C, dtype=np.float32) * SCALE
    o = kernel(x, w_qkv, w_conv1)
    print("out", o.shape, o.dtype, float(np.abs(o).max()))
